# revision 20
# baseline (speedup 1.0000x reference)
"""Multi-head attention (B=4, S=2048, D=1024, H=16, causal) on 8 trn2 cores.

Sharding: core = b*2 + g  (b = batch 0..3, g = head-group 0..1, 8 heads each).
All matmul operands are bf16 (enables Fast Weight Load on LDWEIGHTS and halves
input DMA); PSUM accumulation stays fp32.

The kernel is emitted as an interleaved software pipeline over 512-wide
sequence chunks so the Tile scheduler can overlap everything:

    A(0) B(0) A(1) C(0) B(1) A(2) C(1) B(2) A(3) C(2) B(3) C(3)

  A(s): Q^T/K^T projections for chunk s (d-major, per-chunk tiles
        QTc/KTc[db][s] = [128 d, 512 s]) and V projection for the four
        128-row key blocks of chunk s (s-major VT[kb] = [128 s, 8 h, 64+1]
        with an appended ones column per head -- the AV matmul then emits
        softmax denominators in PSUM row 64 for free).
  B(qb): for each head pair hp: transposed scores S^T = K_h Q_h^T per 128-key
        block, both heads row-tiled into disjoint PE row groups (concurrent)
        into one [128,2,512] PSUM tile; causal handling via block skipping +
        additive -60 triangle mask matmul; one Exp ACTIVATE per block covers
        both heads; O^T accumulated over key blocks into a [128,2,512] PSUM
        pair drawn from the same 3-slot pool (so the next pair's AV can start
        while the previous normalizes); normalization = PE broadcast of the
        fp32r-rounded denominator row, approx-reciprocal, DVE multiply into
        OTc[hp][qb] (bf16).
  C(qb): out[qb chunk] = OTc^T @ Wo + bo, DVE-staged, DMA to HBM.

Host: input transpose/bf16-cast + shard; the g-pair partial sum (row-parallel
Wo all-reduce) happens at gather time.
"""

import numpy as np

S = 2048
D = 1024
DL = 512          # local head dims per core (8 heads x 64)
HL = 8            # local heads
DK = 64
NBK = D // 128    # contraction tiles for projections
NDB = DL // 128   # d-out blocks (head pairs)
NQ = S // 512     # q blocks
NBS = S // 128    # s tiles / key blocks
MASK_VAL = -60.0

_NC = {}


def _build_nc():
    import concourse.bass as bass
    import concourse.mybir as mybir
    import concourse.tile as tile
    from concourse import bacc

    F32 = mybir.dt.float32
    F32R = mybir.dt.float32r
    BF16 = mybir.dt.bfloat16
    Exp = mybir.ActivationFunctionType.Exp

    nc = bacc.Bacc(None)

    xq = nc.dram_tensor("xq", [128, NBK, S], BF16, kind="ExternalInput")
    xk = nc.dram_tensor("xk", [128, NBK, S], BF16, kind="ExternalInput")
    xv = nc.dram_tensor("xv", [128, NBK, S], BF16, kind="ExternalInput")
    wq = nc.dram_tensor("wq", [128, NBK, DL], BF16, kind="ExternalInput")
    wk = nc.dram_tensor("wk", [128, NBK, DL], BF16, kind="ExternalInput")
    wv = nc.dram_tensor("wv", [128, NBK, DL], BF16, kind="ExternalInput")
    wo = nc.dram_tensor("wo", [128, NDB, D], BF16, kind="ExternalInput")
    bqs = nc.dram_tensor("bqs", [128, NDB], F32, kind="ExternalInput")
    bks = nc.dram_tensor("bks", [128, NDB], F32, kind="ExternalInput")
    bvr = nc.dram_tensor("bvr", [1, DL], BF16, kind="ExternalInput")
    bor = nc.dram_tensor("bor", [1, D], BF16, kind="ExternalInput")
    tri = nc.dram_tensor("tri", [128, 128], BF16, kind="ExternalInput")
    idn = nc.dram_tensor("idn", [128, 128], BF16, kind="ExternalInput")
    onesb = nc.dram_tensor("onesb", [1, 128], BF16, kind="ExternalInput")
    onesr = nc.dram_tensor("onesr", [1, DK], F32R, kind="ExternalInput")
    out_d = nc.dram_tensor("out", [S, D], F32, kind="ExternalOutput")

    with tile.TileContext(nc) as tc, nc.allow_low_precision(
            reason="bf16 matmul operands are intended"):
        with (
            tc.tile_pool(name="const", bufs=1) as cpool,
            tc.tile_pool(name="res", bufs=1) as rpool,
            tc.tile_pool(name="xt", bufs=3) as xpool,
            tc.tile_pool(name="pt", bufs=20) as ptpool,
            tc.tile_pool(name="rc", bufs=2) as rcpool,
            tc.tile_pool(name="rb", bufs=2) as rbpool,
            tc.tile_pool(name="ot", bufs=3) as otpool,
            tc.tile_pool(name="mp", bufs=2, space="PSUM") as mpool,
            tc.tile_pool(name="bp", bufs=3, space="PSUM") as bpool,
        ):
            # Startup order matters: the first projection only needs
            # bqs/bks/bv + wq + the first xq chunk, so issue those DMAs first
            # and defer the B/C-phase constants (tri, wo, ...) until after the
            # first A chunk is emitted.
            bqs_sb = cpool.tile([128, NDB], F32, name="bqs", tag="bqs")
            bks_sb = cpool.tile([128, NDB], F32, name="bks", tag="bks")
            bv_sb = cpool.tile([1, DL], BF16, name="bv", tag="bv")
            tri_sb = cpool.tile([128, 128], BF16, name="tri", tag="tri")
            id_sb = cpool.tile([128, 128], BF16, name="idn", tag="idn")
            onesb_sb = cpool.tile([1, 128], BF16, name="onesb", tag="onesb")
            onesr_sb = cpool.tile([1, DK], F32R, name="onesr", tag="onesr")
            bo_sb = cpool.tile([1, D], BF16, name="bo", tag="bo")
            for t, dt_ in [(bqs_sb, bqs), (bks_sb, bks), (bv_sb, bvr),
                           (onesb_sb, onesb)]:
                nc.sync.dma_start(t[:], dt_[:])

            wq_sb = rpool.tile([128, NBK, DL], BF16, name="wq", tag="wq")
            wk_sb = rpool.tile([128, NBK, DL], BF16, name="wk", tag="wk")
            wv_sb = rpool.tile([128, NBK, DL], BF16, name="wv", tag="wv")
            wo_sb = rpool.tile([128, NDB, D], BF16, name="wo", tag="wo")

            QTc = [[rpool.tile([128, 512], BF16, name=f"QT{i}_{s}", tag=f"QT{i}_{s}")
                    for s in range(NQ)] for i in range(NDB)]
            KTc = [[rpool.tile([128, 512], BF16, name=f"KT{i}_{s}", tag=f"KT{i}_{s}")
                    for s in range(NQ)] for i in range(NDB)]
            # VT is flat [128, 583] so the AV stationary operand can be read as
            # a full 128-column window starting at head h's V block (the extra
            # 63 columns spill into head h+1 / zero padding; the resulting
            # PSUM rows 65..127 are never read). 128 bf16 weight columns
            # enable Fast Weight Load.
            VT = [rpool.tile([128, HL * (DK + 1) + 63], BF16,
                             name=f"VT{i}", tag=f"VT{i}")
                  for i in range(NBS)]
            OTc = [[rpool.tile([128, 512], BF16, name=f"OT{i}_{s}", tag=f"OT{i}_{s}")
                    for s in range(NQ)] for i in range(NDB)]

            def phase_a(s, first=False):
                # DMAs first; then compute head-pair 0 of Q/K and all of V
                # before head-pairs 1..3, so B(qb=s, hp=0) unblocks early.
                sl = slice(s * 512, (s + 1) * 512)
                xts = []
                for xd, w_sb_, wd in ((xq, wq_sb, wq), (xk, wk_sb, wk),
                                      (xv, wv_sb, wv)):
                    if first:
                        nc.sync.dma_start(w_sb_[:], wd[:])
                    xt = xpool.tile([128, NBK, 512], BF16, name="xt", tag="xt")
                    nc.sync.dma_start(xt[:], xd[:, :, sl])
                    xts.append(xt)

                def proj_db(xt, w_sb_, b_sb, dst, db):
                    ps = mpool.tile([128, 512], F32, name="mp", tag="mp")
                    for k in range(NBK):
                        nc.tensor.matmul(
                            ps[:], w_sb_[:, k, db * 128:(db + 1) * 128],
                            xt[:, k, :], start=(k == 0), stop=(k == NBK - 1))
                    nc.vector.tensor_scalar_add(
                        dst[db][s][:], ps[:], b_sb[:, db:db + 1])

                proj_db(xts[0], wq_sb, bqs_sb, QTc, 0)
                proj_db(xts[1], wk_sb, bks_sb, KTc, 0)
                for mi in range(4):
                    m = 4 * s + mi
                    ps = mpool.tile([128, 512], F32, name="mp", tag="mp")
                    for k in range(NBK):
                        nc.tensor.matmul(
                            ps[:], xts[2][:, k, mi * 128:(mi + 1) * 128],
                            wv_sb[:, k, :], start=(k == 0), stop=False)
                    nc.tensor.matmul(ps[:], onesb_sb[0:1, :], bv_sb[0:1, :],
                                     start=False, stop=True)
                    vt3 = VT[m][:, 0:HL * (DK + 1)].rearrange(
                        "p (h c) -> p h c", c=DK + 1)
                    nc.vector.memset(vt3[:, :, DK:DK + 1], 1.0)
                    nc.vector.memset(VT[m][:, HL * (DK + 1):], 0.0)
                    nc.vector.tensor_copy(
                        vt3[:, :, 0:DK],
                        ps[:].rearrange("p (h c) -> p h c", c=DK))
                for db in range(1, NDB):
                    proj_db(xts[0], wq_sb, bqs_sb, QTc, db)
                    proj_db(xts[1], wk_sb, bks_sb, KTc, db)

            # B runs as a software pipeline over (qb, hp) iterations: while
            # iteration i computes scores+exp (ACT-bound), the AV chain of
            # iteration i-1 -- whose pt tiles are all ready -- streams
            # back-to-back on the PE with no dependency waits.
            bstate = {"prev": None}

            # The norm is split in two: stage 1 (the DVE cast of the
            # denominator row) fires as soon as an AV chain completes; stage 2
            # (broadcast matmuls + reciprocal + multiply) is deferred a couple
            # of key blocks so the broadcast matmul never sits at the head of
            # the PE queue waiting for the cast -- that wait would block the
            # ready score/AV matmuls queued behind it.
            def emit_norm1(it):
                it["den"] = rcpool.tile([1, 2, 512], F32R, name="rc", tag="rc")
                nc.vector.tensor_copy(it["den"][:], it["pso"][DK:DK + 1, :, :])

            def emit_norm2(it):
                hp, qb, pso, den = it["hp"], it["qb"], it["pso"], it["den"]
                rb = rbpool.tile([DK, 2, 512], F32, name="rb", tag="rb")
                for h2 in range(2):
                    psb = mpool.tile([DK, 512], F32, name="mp", tag="mp")
                    nc.tensor.matmul(psb[:], onesr_sb[0:1, :],
                                     den[0:1, h2, :],
                                     start=True, stop=True,
                                     skip_group_check=True)
                    nc.vector.reciprocal_approx_fast(
                        out=rb[:, h2, :], in_=psb[:])
                for h2 in range(2):
                    nc.vector.tensor_mul(
                        OTc[hp][qb][h2 * DK:(h2 + 1) * DK, :],
                        pso[0:DK, h2, :], rb[:, h2, :])

            def flush_norm2():
                it = bstate.get("norm2")
                if it is not None:
                    emit_norm2(it)
                    bstate["norm2"] = None

            def emit_avs(it, n):
                """Emit AV matmuls for iteration `it` up to index n; when the
                chain completes, normalize immediately so the pso slot frees
                as early as possible."""
                if it is None or it.get("done"):
                    return
                while it["emitted"] < min(n, it["kbmax"]):
                    kb, pt_, minq = it["pts"][it["emitted"]]
                    if it["pso"] is None:
                        it["pso"] = bpool.tile([128, 2, 512], F32,
                                               name="bp", tag="bp")
                    for h2 in range(2):
                        h = it["hp"] * 2 + h2
                        nc.tensor.matmul(
                            it["pso"][0:128, h2, minq:512],
                            VT[kb][:, h * (DK + 1):h * (DK + 1) + 128],
                            pt_[:, h2, minq:512],
                            start=(kb == 0), stop=(kb == it["kbmax"] - 1),
                            skip_group_check=True)
                    it["emitted"] += 1
                if it["emitted"] == it["kbmax"]:
                    it["done"] = True
                    emit_norm1(it)
                    flush_norm2()
                    bstate["norm2"] = it
                    bstate["norm2_age"] = 0

            def b_drain():
                it = bstate["prev"]
                bstate["prev"] = None
                emit_avs(it, 1 << 30)

            def b_iter(qb, hp, last=False):
                kbmax = 4 * (qb + 1)
                cur = {"hp": hp, "qb": qb, "kbmax": kbmax,
                       "pts": [], "pso": None, "emitted": 0}
                for kb in range(kbmax):
                    # Ready AV matmuls go in front of the score matmul: the
                    # score may wait on a PSUM slot (exp pacing) and the PE
                    # queue is strict FIFO.
                    emit_avs(bstate["prev"], kb + 2)
                    if bstate.get("norm2") is not None:
                        bstate["norm2_age"] += 1
                        if bstate["norm2_age"] >= 2:
                            flush_norm2()
                    di = kb - 4 * qb
                    minq = 128 * di if di > 0 else 0
                    pss = bpool.tile([128, 2, 512], F32, name="bp", tag="bp")
                    for h2 in range(2):
                        base = h2 * DK
                        nc.tensor.matmul(
                            pss[:, h2, minq:512],
                            KTc[hp][kb // 4][base:base + DK,
                                             (kb % 4) * 128:(kb % 4 + 1) * 128],
                            QTc[hp][qb][base:base + DK, minq:512],
                            start=True, stop=(di < 0),
                            skip_group_check=True)
                    if di >= 0:
                        for h2 in range(2):
                            nc.tensor.matmul(
                                pss[:, h2, minq:minq + 128], id_sb[:], tri_sb[:],
                                start=False, stop=True,
                                skip_group_check=True)
                    pt_ = ptpool.tile([128, 2, 512], BF16, name="pt", tag="pt")
                    nc.scalar.activation(pt_[:, :, minq:512],
                                         pss[:, :, minq:512], Exp)
                    cur["pts"].append((kb, pt_, minq))
                    if last:
                        emit_avs(cur, kb)  # self-AVs trail exp by one block
                b_drain()
                if last:
                    emit_avs(cur, 1 << 30)
                    flush_norm2()
                else:
                    bstate["prev"] = cur

            def phase_c(qb):
                for mi in range(4):
                    m = 4 * qb + mi
                    msl = slice(m * 128, (m + 1) * 128)
                    for n2 in range(2):
                        nsl = slice(n2 * 512, (n2 + 1) * 512)
                        ps = mpool.tile([128, 512], F32, name="mp", tag="mp")
                        for db in range(NDB):
                            nc.tensor.matmul(
                                ps[:], OTc[db][qb][:, mi * 128:(mi + 1) * 128],
                                wo_sb[:, db, nsl],
                                start=(db == 0), stop=False)
                        nc.tensor.matmul(ps[:], onesb_sb[0:1, :], bo_sb[0:1, nsl],
                                         start=False, stop=True)
                        ot = otpool.tile([128, 512], F32, name="ob", tag="ob")
                        nc.vector.tensor_copy(ot[:], ps[:])
                        nc.sync.dma_start(out_d[msl, nsl], ot[:])

            phase_a(0, first=True)
            for t, dt_ in [(tri_sb, tri), (id_sb, idn), (onesr_sb, onesr),
                           (bo_sb, bor), (wo_sb, wo)]:
                nc.sync.dma_start(t[:], dt_[:])
            for hp in range(NDB):
                b_iter(0, hp)
            phase_a(1)
            b_iter(1, 0)          # drains+norms (0,3) -> OTc[*][0] complete
            phase_c(0)
            for hp in range(1, NDB):
                b_iter(1, hp)
            phase_a(2)
            b_iter(2, 0)
            phase_c(1)
            for hp in range(1, NDB):
                b_iter(2, hp)
            phase_a(3)
            b_iter(3, 0)
            phase_c(2)
            b_iter(3, 1)
            b_iter(3, 2)
            b_iter(3, 3, last=True)
            phase_c(3)

    nc.finalize()
    return nc


def _to_pkt(a2d, nt):
    """[nt*128, N] -> [128, nt, N] (partition-major tiling of the first dim)."""
    n = a2d.shape[1]
    return np.ascontiguousarray(
        a2d.reshape(nt, 128, n).transpose(1, 0, 2))


def _make_in_maps(query, value, key, Wq, bq, Wk, bk, Wv, bv, Wo, bo):
    import ml_dtypes

    f32 = np.float32
    bf16 = ml_dtypes.bfloat16
    query = np.asarray(query, f32)
    value = np.asarray(value, f32)
    key = np.asarray(key, f32)
    Wq = np.asarray(Wq, f32); bq = np.asarray(bq, f32)
    Wk = np.asarray(Wk, f32); bk = np.asarray(bk, f32)
    Wv = np.asarray(Wv, f32); bv = np.asarray(bv, f32)
    Wo = np.asarray(Wo, f32); bo = np.asarray(bo, f32)

    p = np.arange(128)[:, None]
    j = np.arange(128)[None, :]
    tri = np.where(p > j, MASK_VAL, 0.0).astype(bf16)
    idn = np.eye(128, dtype=bf16)
    onesb = np.ones((1, 128), bf16)
    onesr = np.ones((1, DK), f32)

    xT = {}
    for nm, x in (("q", query), ("k", key), ("v", value)):
        xT[nm] = [_to_pkt(x[b].T.astype(bf16), NBK) for b in range(4)]

    in_maps = []
    for b in range(4):
        for g in range(2):
            sl = slice(g * DL, (g + 1) * DL)
            m = {
                "xq": xT["q"][b],
                "xk": xT["k"][b],
                "xv": xT["v"][b],
                "wq": _to_pkt((Wq[:, sl] / 8.0).astype(bf16), NBK),
                "wk": _to_pkt(Wk[:, sl].astype(bf16), NBK),
                "wv": _to_pkt(Wv[:, sl].astype(bf16), NBK),
                "wo": _to_pkt(Wo[sl, :].astype(bf16), NDB),
                "bqs": np.ascontiguousarray((bq[sl] / 8.0).reshape(NDB, 128).T),
                "bks": np.ascontiguousarray(bk[sl].reshape(NDB, 128).T),
                "bvr": bv[sl].reshape(1, DL).astype(bf16),
                "bor": (bo if g == 0 else np.zeros_like(bo)).reshape(1, D).astype(bf16),
                "tri": tri, "idn": idn, "onesb": onesb, "onesr": onesr,
            }
            in_maps.append(m)
    return in_maps


def kernel_with_info(inputs, trace=False):
    from concourse.bass_utils import run_bass_kernel_spmd

    if "nc" not in _NC:
        _NC["nc"] = _build_nc()

    in_maps = _make_in_maps(**inputs)
    res = run_bass_kernel_spmd(_NC["nc"], in_maps, core_ids=list(range(8)),
                               trace=trace)
    out = np.empty((4, S, D), np.float32)
    for b in range(4):
        out[b] = res.results[2 * b]["out"] + res.results[2 * b + 1]["out"]
    return out, res


def kernel(**inputs):
    out, _ = kernel_with_info(inputs)
    return out
